# revision 17
# baseline (speedup 1.0000x reference)
"""Trainium2 Bass kernel for nn_CCN_63299228009054 (gnn_message_passing).

Data-parallel over batch: 8 NeuronCores x 8 batches each. Per core:
  - PE computes approximate -dist^2 via a K=4 Gram matmul into PSUM.
  - DVE max/max_index extracts top-8 candidate neighbors per node row.
  - Exact re-rank: indirect-DMA gather of candidate coords, recompute
    d = sqrt((xi-xj)^2 + (yi-yj)^2) with IEEE fp32 ops, rank candidates
    by (d, idx) via a compare matrix (matches jax.lax.top_k ties).
  - Neighbor features: tables Hk = F2[0] @ Wnb_k^T (k = rank slot) in
    DRAM; one indirect DMA gathers rank-relabeled rows (ranks 6,7 hit a
    zero table); PE identity-matmuls sum the 8 slots and accumulate
    F3 + bias in PSUM; leaky-relu on DVE; mean via ones-column matmul.
"""

import numpy as np
from contextlib import ExitStack

import concourse.bass as bass
import concourse.mybir as mybir
from concourse.bass import IndirectOffsetOnAxis
from concourse.bass_types import AP
from concourse.tile import TileContext
from concourse import bass_utils
from concourse.library_config import mlp as MLP_LIB

F32 = mybir.dt.float32
U16 = mybir.dt.uint16
U32 = mybir.dt.uint32
ALU = mybir.AluOpType
ACTF = mybir.ActivationFunctionType
AXL = mybir.AxisListType

B, N, D, K = 64, 1024, 128, 6
NCORES = 8
BPC = B // NCORES          # batches per core
NBLK = N // 128            # node blocks per batch
NC8 = 8                    # candidates per node
NTAB = 7                   # 6 rank tables + zero table


def rap(t, extra_offset, pairs):
    """Raw AP on a tile/tensor with explicit [step, count] pairs (elements)."""
    off = (t.offset if isinstance(t, AP) else 0) + extra_offset
    tensor = t.tensor if isinstance(t, AP) else t
    return AP(tensor, off, [list(p) for p in pairs])


def dma_gather_raw(nc, out_ap, in_ap, idxs_ap, num_idxs, elem_size, elem_step):
    """nc.gpsimd.dma_gather minus the elem_size%256B assert (elem_step stride
    must still be a 256B multiple). HBM source, non-transpose only."""
    import concourse.mybir as _mb
    from concourse import ap_utils as _apu
    gp = nc.gpsimd
    assert idxs_ap.dtype == _mb.dt.int16
    assert in_ap.dtype == out_ap.dtype
    stride_bytes = elem_step * _mb.dt.size(in_ap.dtype)
    assert stride_bytes % 256 == 0
    stride_bytes_256 = stride_bytes // 256
    _in_ap = gp.lower_ap_dma(in_ap, for_custom_bir_dma=True)
    _idxs_ap = gp.lower_ap(idxs_ap)
    _out_ap = gp.lower_ap(out_ap)
    return gp.add_instruction(
        _mb.InstDMAGatherAnt(
            name=nc.get_next_instruction_name(),
            ins=[*_in_ap, _idxs_ap, gp.lower_val_access(gp.to_reg(num_idxs))],
            outs=[_out_ap],
            transpose=False,
            num_idxs=num_idxs,
            elem_size=elem_size,
            stride_bytes_256=stride_bytes_256,
            gen_mode=0,
            single_packet=True,
            queue_num=0,
            sbuf_tokens_per_rank=0,
            sbuf_free_dim_per_rank=0,
            sbuf_free_dim_pad_per_rank=0,
            sbuf_byte_offset=0,
        ))


def split_sync_waits(nc, max_waits=1):
    """Walrus codegen limits sem waits per instruction (TPB_CTRL wait slots;
    Drain takes none). Hoist excess waits onto same-engine NoOps placed just
    before the offending instruction (engine queues are in-order, so
    semantics are preserved)."""
    cnt = 0
    for f in nc.m.functions:
        for b in f.blocks:
            insts = b.instructions
            out = []
            changed = False
            for i in insts:
                limit = 0 if isinstance(i, mybir.InstDrain) else max_waits
                si = i.sync_info
                waits = list(si.on_wait) if (si is not None and si.on_wait) else []
                if len(waits) > limit:
                    changed = True
                    extra, keep = (waits, []) if limit == 0 else (waits[:-limit], waits[-limit:])
                    for j in range(0, len(extra), max_waits):
                        cnt += 1
                        nop = mybir.InstNoOp(
                            name=f"W-split-{cnt}", ins=[], outs=[],
                            sync_info=mybir.SyncInfo(
                                on_wait=extra[j:j + max_waits], on_update=[]))
                        nop.engine = i.engine
                        nc.register_instruction(nop)
                        out.append(nop)
                    si.on_wait = keep
                out.append(i)
            if changed:
                b.instructions = out
    return cnt


def build_program(bpc=BPC):
    nc = bass.Bass("TRN2", enable_asserts=False, debug=False)

    def din(name, shape):
        return nc.dram_tensor(name, list(shape), F32, kind="ExternalInput").ap()

    loc_b = [din(f"loc_b{i}", (N, 2)) for i in range(bpc)]
    lhsT_gram = din("lhsT_gram", (bpc, 4, N))   # rows: x, y, s=x^2+y^2, ones
    rhs_gram = din("rhs_gram", (bpc, 4, N))     # rows: 2x, 2y, -ones, -s
    xdlT = din("xdlT", (bpc, 3, N))             # rows: x, y, deadline
    xi_col = din("xi_col", (128, bpc * NBLK))
    yi_col = din("yi_col", (128, bpc * NBLK))
    loc0T = din("loc0T", (2, N))
    w2dT = din("w2dT", (2, D))
    b2d_col = din("b2d_col", (D, 1))
    wnbT = din("wnbT", (K * D, D))
    w3dT = din("w3dT", (3, D))
    btot = din("btot", (1, D))
    wdepT = din("wdepT", (2, D))
    bdep = din("bdep", (1, D))
    depotT = din("depotT", (2, bpc))
    ones_row = din("ones_row", (1, 128))
    ones_col = din("ones_col", (128, 1))
    ident = din("ident", (128, 128))

    out_h = nc.dram_tensor("out_h", [bpc, N + 1, D], F32, kind="ExternalOutput").ap()
    out_mean = nc.dram_tensor("out_mean", [bpc, D], F32, kind="ExternalOutput").ap()
    htab = nc.dram_tensor("htab", [NTAB * N, D], F32, kind="Internal").ap()

    with TileContext(nc) as tc, ExitStack() as ctx:
        cpool = ctx.enter_context(tc.tile_pool(name="consts", bufs=1))
        spool = ctx.enter_context(tc.tile_pool(name="work", bufs=2))
        gpool = ctx.enter_context(tc.tile_pool(name="gather", bufs=2))
        ppool = ctx.enter_context(tc.tile_pool(name="psum", bufs=2, space="PSUM"))
        hpool = ctx.enter_context(tc.tile_pool(name="psumh", bufs=2, space="PSUM"))
        mpool = ctx.enter_context(tc.tile_pool(name="psumm", bufs=1, space="PSUM"))

        def load(tag, ap_in, shape=None, pairs=None):
            t = cpool.tile(shape or list(ap_in.shape), F32, tag=tag)
            nc.sync.dma_start(out=t, in_=ap_in if pairs is None else rap(ap_in, 0, pairs))
            return t

        ident_sb = load("ident", ident)
        ones_sb = load("ones", ones_row)
        onesc_sb = load("onesc", ones_col)
        w2dT_sb = load("w2dT", w2dT)
        loc0T_sb = load("loc0T", loc0T)
        b2d_sb = load("b2d", b2d_col)
        w3dT_sb = load("w3dT", w3dT)
        btot_sb = load("btot", btot)
        wdepT_sb = load("wdepT", wdepT)
        bdep_sb = load("bdep", bdep)
        depotT_sb = load("depotT", depotT)
        xi_sb = load("xi", xi_col)
        yi_sb = load("yi", yi_col)
        lg_sb = load("lg", lhsT_gram, [4, bpc, N], [[N, 4], [4 * N, bpc], [1, N]])
        rg_sb = load("rg", rhs_gram, [4, bpc, N], [[N, 4], [4 * N, bpc], [1, N]])
        xdl_sb = load("xdl", xdlT, [3, bpc, N], [[N, 3], [3 * N, bpc], [1, N]])
        wnbT_sb = load("wnbT", wnbT, [128, K, D], [[D, 128], [128 * D, K], [1, D]])

        # ---- F2[0]^T = W2d @ loc0^T + b2d : [D, N] ----
        f20_ps = ppool.tile([128, N], F32, tag="bigps")
        for h in range(2):
            nc.tensor.matmul(f20_ps[:, h * 512:(h + 1) * 512], w2dT_sb,
                             loc0T_sb[:, h * 512:(h + 1) * 512], start=True, stop=True)
        f20_sb = cpool.tile([128, N], F32)
        nc.scalar.activation(f20_sb, f20_ps, ACTF.Identity, bias=b2d_sb, scale=1.0)

        # ---- Hk tables -> DRAM ----
        for k in range(K):
            for blk in range(NBLK):
                hk_ps = hpool.tile([128, D], F32, tag="h128")
                nc.tensor.matmul(hk_ps, f20_sb[:, blk * 128:(blk + 1) * 128],
                                 wnbT_sb[:, k, :], start=True, stop=True)
                hk_sb = spool.tile([128, D], F32, tag="hout")
                nc.scalar.activation(hk_sb, hk_ps, ACTF.Copy, bias=0.0, scale=1.0)
                nc.sync.dma_start(
                    out=rap(htab, (k * N + blk * 128) * D, [[D, 128], [1, D]]),
                    in_=hk_sb)
        zt = spool.tile([128, 512], F32, tag="zt")
        nc.vector.memset(zt, 0.0)
        for rep in range(2):
            nc.sync.dma_start(
                out=rap(htab, (K * N) * D + rep * 512 * 128, [[512, 128], [1, 512]]),
                in_=zt)

        # ---- depot rows ----
        dep_ps = mpool.tile([bpc, D], F32, tag="dep")
        nc.tensor.matmul(dep_ps, depotT_sb, wdepT_sb, start=True, stop=False)
        nc.tensor.matmul(dep_ps, ones_sb[:, :bpc], bdep_sb, start=False, stop=True)
        dep_sb0 = cpool.tile([bpc, D], F32)
        nc.scalar.activation(dep_sb0, dep_ps, ACTF.Copy, bias=0.0, scale=1.0)
        dep_sb = cpool.tile([bpc, D], F32)
        nc.vector.scalar_tensor_tensor(dep_sb, dep_sb0, 0.01, dep_sb0,
                                       op0=ALU.mult, op1=ALU.max)
        for b in range(bpc):
            nc.sync.dma_start(out=rap(out_h, b * (N + 1) * D, [[D, 1], [1, D]]),
                              in_=dep_sb[b:b + 1, :])

        # ---- per-batch main loop ----
        for b in range(bpc):
            idx8 = spool.tile([128, NBLK, NC8], U16, tag="idx8")
            for blk in range(NBLK):
                nsq_ps = ppool.tile([128, N], F32, tag="bigps")
                lt = lg_sb[:, b, blk * 128:(blk + 1) * 128]
                for h in range(2):
                    nc.tensor.matmul(nsq_ps[:, h * 512:(h + 1) * 512], lt,
                                     rg_sb[:, b, h * 512:(h + 1) * 512],
                                     start=True, stop=True)
                vals8 = spool.tile([128, NC8], F32, tag="vals8")
                nc.vector.max(out=vals8, in_=nsq_ps)
                nc.vector.max_index(out=idx8[:, blk, :], in_max=vals8, in_values=nsq_ps)

            # ---- exact re-rank ----
            # wrap idx8 into dma_gather index layout: ordinal o = c*1024+node;
            # value at wrapped[o%16, o//16]; scramble via 2B-granule DMA then
            # replicate partitions 0-15 to all 8 groups.
            NF = NBLK * NC8
            idx8f = idx8.rearrange("p a b -> p (a b)")
            idxf = spool.tile([128, NF], F32, tag="idxf")
            nc.vector.tensor_copy(idxf, idx8f)
            idx32 = spool.tile([128, NF], U32, tag="idx32")
            nc.vector.tensor_copy(idx32, idxf)
            xyc = spool.tile([128, NF, 2], F32, tag="xyc")
            for f in range(NF):
                nc.gpsimd.indirect_dma_start(
                    out=xyc[:, f, :], out_offset=None, in_=loc_b[b],
                    in_offset=IndirectOffsetOnAxis(ap=idx32[:, f:f + 1], axis=0))
            # xyc slot = blk*8+c; view as [p, blk, c]
            xc = rap(xyc, 0, [[NF * 2, 128], [2 * NC8, NBLK], [2, NC8]])
            yc = rap(xyc, 1, [[NF * 2, 128], [2 * NC8, NBLK], [2, NC8]])
            xi_bc = rap(xi_sb, b * NBLK, [[bpc * NBLK, 128], [1, NBLK], [0, NC8]])
            yi_bc = rap(yi_sb, b * NBLK, [[bpc * NBLK, 128], [1, NBLK], [0, NC8]])
            dx = spool.tile([128, NBLK, NC8], F32, tag="dx")
            nc.vector.tensor_tensor(dx, xc, xi_bc, ALU.subtract)
            dy = spool.tile([128, NBLK, NC8], F32, tag="dy")
            nc.vector.tensor_tensor(dy, yc, yi_bc, ALU.subtract)
            d2 = spool.tile([128, NBLK, NC8], F32, tag="d2")
            nc.vector.tensor_tensor(d2, dx, dx, ALU.mult)
            dy2 = spool.tile([128, NBLK, NC8], F32, tag="dy2")
            nc.vector.tensor_tensor(dy2, dy, dy, ALU.mult)
            nc.vector.tensor_tensor(d2, d2, dy2, ALU.add)
            dd = spool.tile([128, NBLK, NC8], F32, tag="dd")
            nc.scalar.activation(dd, d2, ACTF.Sqrt, bias=0.0, scale=1.0)

            # rank_c = sum_c' [ (d_c' < d_c) or (d_c' == d_c and j_c' < j_c) ]
            def bc_c(t):    # indexed by c (3rd axis), broadcast over c' (4th)
                return rap(t, 0, [[NF, 128], [NC8, NBLK], [1, NC8], [0, NC8]])

            def bc_cp(t):   # indexed by c' (4th axis), broadcast over c (3rd)
                return rap(t, 0, [[NF, 128], [NC8, NBLK], [0, NC8], [1, NC8]])

            m1 = spool.tile([128, NBLK, NC8, NC8], F32, tag="m1")
            nc.vector.tensor_tensor(m1, bc_cp(dd), bc_c(dd), ALU.is_lt)
            m2 = spool.tile([128, NBLK, NC8, NC8], F32, tag="m2")
            nc.vector.tensor_tensor(m2, bc_cp(dd), bc_c(dd), ALU.is_equal)
            m3 = spool.tile([128, NBLK, NC8, NC8], F32, tag="m3")
            nc.vector.tensor_tensor(m3, bc_cp(idxf), bc_c(idxf), ALU.is_lt)
            nc.vector.tensor_tensor(m2, m2, m3, ALU.logical_and)
            nc.vector.tensor_tensor(m1, m1, m2, ALU.add)
            rank = spool.tile([128, NBLK, NC8], F32, tag="rank")
            nc.vector.tensor_reduce(rank, m1, AXL.X, ALU.add)

            nc.vector.tensor_scalar_min(rank, rank, 6.0)
            offs = spool.tile([128, NF], F32, tag="offs")
            nc.vector.scalar_tensor_tensor(
                offs, rank.rearrange("p a b -> p (a b)"), float(N), idxf,
                op0=ALU.mult, op1=ALU.add)
            offs32 = spool.tile([128, NF], U32, tag="offs32")
            nc.vector.tensor_copy(offs32, offs)
            g_sb = gpool.tile([128, NF, D], F32, tag="gath")
            for f in range(NF):
                nc.gpsimd.indirect_dma_start(
                    out=g_sb[:, f, :], out_offset=None, in_=htab,
                    in_offset=IndirectOffsetOnAxis(ap=offs32[:, f:f + 1], axis=0))

            # ---- h blocks ----
            mean_ps = mpool.tile([1, D], F32, tag="mean")
            for blk in range(NBLK):
                h_ps = hpool.tile([128, D], F32, tag="h128")
                for c in range(NC8):
                    nc.tensor.matmul(h_ps, ident_sb, g_sb[:, blk * NC8 + c, :],
                                     start=(c == 0), stop=False)
                nc.tensor.matmul(h_ps, xdl_sb[:, b, blk * 128:(blk + 1) * 128],
                                 w3dT_sb, start=False, stop=False)
                nc.tensor.matmul(h_ps, ones_sb, btot_sb, start=False, stop=True)
                h_sb0 = spool.tile([128, D], F32, tag="hout0")
                nc.scalar.activation(h_sb0, h_ps, ACTF.Copy, bias=0.0, scale=1.0)
                h_sb = spool.tile([128, D], F32, tag="hout")
                nc.vector.scalar_tensor_tensor(h_sb, h_sb0, 0.01, h_sb0,
                                               op0=ALU.mult, op1=ALU.max)
                nc.sync.dma_start(
                    out=rap(out_h, (b * (N + 1) + 1 + blk * 128) * D,
                            [[D, 128], [1, D]]),
                    in_=h_sb)
                nc.tensor.matmul(mean_ps, onesc_sb, h_sb,
                                 start=(blk == 0), stop=False)
            nc.tensor.matmul(mean_ps, ident_sb[:bpc, b:b + 1], dep_sb,
                             start=False, stop=True)
            mean_sb = spool.tile([1, D], F32, tag="meansb")
            nc.scalar.activation(mean_sb, mean_ps, ACTF.Copy, bias=0.0,
                                 scale=1.0 / float(N + 1))
            nc.sync.dma_start(out=rap(out_mean, b * D, [[D, 1], [1, D]]), in_=mean_sb)

    n = split_sync_waits(nc)
    if n:
        print(f"split_sync_waits: inserted {n} wait NoOps")
    return nc


def make_in_maps(loc, deadline, depot, W3d, b3d, W2d, b2d, Wnb, bnb, Wdep, bdep,
                 bpc=BPC, ncores=NCORES):
    f = np.float32
    loc = np.ascontiguousarray(loc, dtype=f)
    deadline = np.ascontiguousarray(deadline, dtype=f)
    depot = np.ascontiguousarray(depot, dtype=f)
    in_maps = []
    ones1024 = np.ones((N,), dtype=f)
    for c in range(ncores):
        lc = loc[c * bpc:(c + 1) * bpc]
        dl = deadline[c * bpc:(c + 1) * bpc]
        x, y = lc[:, :, 0], lc[:, :, 1]
        s = (x * x + y * y).astype(f)
        ones_b = np.broadcast_to(ones1024, x.shape)
        xr = x.reshape(bpc, NBLK, 128)
        yr = y.reshape(bpc, NBLK, 128)
        m = {
            "lhsT_gram": np.ascontiguousarray(np.stack([x, y, s, ones_b], axis=1), dtype=f),
            "rhs_gram": np.ascontiguousarray(np.stack([2 * x, 2 * y, -ones_b, -s], axis=1), dtype=f),
            "xdlT": np.ascontiguousarray(np.stack([x, y, dl], axis=1), dtype=f),
            "xi_col": np.ascontiguousarray(xr.transpose(2, 0, 1).reshape(128, bpc * NBLK)),
            "yi_col": np.ascontiguousarray(yr.transpose(2, 0, 1).reshape(128, bpc * NBLK)),
            "loc0T": np.ascontiguousarray(loc[0].T),
            "w2dT": np.ascontiguousarray(W2d.T, dtype=f),
            "b2d_col": np.ascontiguousarray(b2d.reshape(D, 1), dtype=f),
            "wnbT": np.ascontiguousarray(Wnb.T, dtype=f),
            "w3dT": np.ascontiguousarray(W3d.T, dtype=f),
            "btot": (b3d + bnb).reshape(1, D).astype(f),
            "wdepT": np.ascontiguousarray(Wdep.T, dtype=f),
            "bdep": np.ascontiguousarray(bdep.reshape(1, D), dtype=f),
            "depotT": np.ascontiguousarray(depot[c * bpc:(c + 1) * bpc].T),
            "ones_row": np.ones((1, 128), dtype=f),
            "ones_col": np.ones((128, 1), dtype=f),
            "ident": np.eye(128, dtype=f),
        }
        for i in range(bpc):
            m[f"loc_b{i}"] = np.ascontiguousarray(lc[i])
        in_maps.append(m)
    return in_maps


_CACHE = {}


def kernel(loc, deadline, depot, W3d, b3d, W2d, b2d, Wnb, bnb, Wdep, bdep):
    if "nc" not in _CACHE:
        _CACHE["nc"] = build_program()
    nc = _CACHE["nc"]
    in_maps = make_in_maps(loc, deadline, depot, W3d, b3d, W2d, b2d,
                           Wnb, bnb, Wdep, bdep)
    res = bass_utils.run_bass_kernel_spmd(nc, in_maps, core_ids=list(range(NCORES)))
    outs = res.results
    h = np.concatenate([o["out_h"] for o in outs], axis=0)
    mean = np.concatenate([o["out_mean"] for o in outs], axis=0)
    return h, mean


if __name__ == "__main__":
    print("building program...")
    nc = build_program()
    print("program built ok")
